# revision 17
# baseline (speedup 1.0000x reference)
"""Trainium2 Bass kernel for a GPT-2 transformer block (nn_Block_29343216566701).

Sharding: data-parallel over batch B=8 -> 8 NeuronCores, one batch element per
core, no collectives. Each core runs the full block on [1024 tokens, 768 feats].

On-chip layout is feature-major (x^T: [768, 1024] as [128, 6, 1024] SBUF tiles)
so every matmul contracts over the partition dim without transposes. x / x1 are
kept in fp32r so LN stat matmuls (jr all-ones/C stationary) read them directly.
LayerNorm1 is half-split and pipelined with the x DMA; LayerNorm2 stats are
fused into the proj loop. Attention softmax denominators: the +ones column of V
produces den at PSUM row 64; per pair both dens are broadcast into one [128,T]
PSUM tile (rows 0:64 / 64:128), one Ln + one Exp gives rec = 1/den for both
heads, applied by two cross-partition-base DVE mults writing yT directly.
proj/fc2 evictions are single fused scalar_tensor_tensor ops (bias+residual).

Precision: the large GEMMs (QKV, proj, fc1, fc2) run in fp8-e4m3 DoubleRow
mode (2 contraction rows/cycle on PE). Weights are scaled x64 host-side so
they quantize in e4m3's normal range; the x64 is carried through the residual
spine (x, x1, out are 64x their true values on chip) rather than descaled per
matmul: LayerNorm is scale-invariant (eps is pre-scaled by 64^2), eviction
biases are host-scaled x64, and the final output is divided by 64 on the host.
Activations feeding fp8 GEMMs (ln out, attn out yT, gelu out h2) are written
as e4m3 directly by their eviction ops at true scale; the PSUM carries
64*(W@a), descaled only where a true-scale result is needed (QKV eviction via
dual-scalar tensor_scalar, V/fc1 eviction via ACT scale=1/64). Attention
scores/softmax/AV stay bf16 with f32 PSUM. LN gamma/beta folded into the
following weight matrix host-side (exact).
"""
import numpy as np
import ml_dtypes

import concourse.bass as bass
import concourse.tile as tile
import concourse.mybir as mybir
from concourse.bass_utils import run_bass_kernel_spmd
from concourse.vector_clock import ScopedClock

F32 = mybir.dt.float32
F32R = mybir.dt.float32r
BF16 = mybir.dt.bfloat16
F8 = mybir.dt.float8e4
DR = mybir.MatmulPerfMode.DoubleRow
AF = mybir.ActivationFunctionType
OP = mybir.AluOpType
BF = ml_dtypes.bfloat16
E4 = ml_dtypes.float8_e4m3
WS = 64.0              # weight/residual fp8 scale (power of 2, exact)

B, T, C = 8, 1024, 768
H, HD = 12, 64
NC = C // 128          # 6 feature chunks
NT = T // 128          # 8 token tiles
FF = 4 * C             # 3072
NF = FF // 128         # 24
LN_EPS = 1e-5


# ---------------------------------------------------------------------------
# walrus codegen accepts only one fused semaphore wait per instruction; hoist
# excess waits onto preceding nofuse NOPs on the same engine.
def _split_excess_waits(nc, cap=1):
    for fn in nc.m.functions:
        for bb in fn.blocks:
            new = []
            changed = False
            for ins in bb.instructions:
                si = getattr(ins, "sync_info", None)
                waits = list(si.on_wait) if (si is not None and si.on_wait) else []
                if len(waits) > cap:
                    changed = True
                    for i, w in enumerate(waits[:-cap]):
                        new.append(mybir.InstNoOp(
                            name=f"{ins.name}-w{i}",
                            engine=ins.engine,
                            sync_info=mybir.SyncInfo(on_wait=[w], on_update=[]),
                            bass_nofuse=True,
                        ))
                    ins.sync_info = mybir.SyncInfo(
                        on_wait=waits[-cap:], on_update=list(si.on_update))
                new.append(ins)
            if changed:
                bb.instructions = new


class _SplitDrainTC(tile.TileContext):
    """TileContext whose kernel-tail drain carries its waits on single-wait
    NOPs (the stock version fuses them all onto one drain instruction)."""

    def _drain_and_barrier(self, tick_clock, wait_clock):
        nc = self.nc
        probe = nc.sync.nop(nofuse=True, hint="tail_wait0")
        wait_clock.add_sem_waits(
            probe.ins, ScopedClock({None: tick_clock.global_clock}))
        waits = list(probe.ins.sync_info.on_wait) if probe.ins.sync_info else []
        if len(waits) > 1:
            probe.ins.sync_info = mybir.SyncInfo(on_wait=waits[:1], on_update=[])
            for i, w in enumerate(waits[1:]):
                n = nc.sync.nop(nofuse=True, hint=f"tail_wait{i + 1}")
                n.ins.sync_info = mybir.SyncInfo(on_wait=[w], on_update=[])
        nc.sync.drain()
        nc.all_engine_barrier()
        assert self.sems is not None
        popped = nc._tile_sem_poison_stack.pop()
        assert popped is self._sem_poison
        nc.clear_and_free_semaphores(list(self.sems.allocated().values()))
        nc.all_engine_barrier()


# ---------------------------------------------------------------------------
PHASE_MARKS = []


def _mark(nc, label):
    if not any(l == label for l, _ in PHASE_MARKS):
        PHASE_MARKS.append((label, len(nc.inst_map)))


def _build(nrep=1, loop_n=0, for_sim=False):
    nc = bass.Bass(trn_type="TRN2", name="gpt2block")

    xT = nc.dram_tensor("xT", [C, T], F32R, kind="ExternalInput")
    wqk = nc.dram_tensor("wqk", [2 * NC, 128, C], F8, kind="ExternalInput")
    wv = nc.dram_tensor("wv", [C, C], F8, kind="ExternalInput")
    bqk = nc.dram_tensor("bqk", [2 * C], F32, kind="ExternalInput")
    bv = nc.dram_tensor("bv", [1, C], BF16, kind="ExternalInput")
    wproj = nc.dram_tensor("wproj", [NC, 128, C], F8, kind="ExternalInput")
    bproj = nc.dram_tensor("bproj", [C], F32, kind="ExternalInput")
    wfc = nc.dram_tensor("wfc", [NF, 128, C], F8, kind="ExternalInput")
    bfc = nc.dram_tensor("bfc", [FF], F32, kind="ExternalInput")
    wfc2 = nc.dram_tensor("wfc2", [NC, 128, FF], F8, kind="ExternalInput")
    bfc2 = nc.dram_tensor("bfc2", [C], F32, kind="ExternalInput")
    outT = nc.dram_tensor("outT", [C, T], F32R, kind="ExternalOutput")

    # causal handling for diagonal 128x128 blocks of s^T[k, q]: accumulate
    # -1e9 where q < k via an identity matmul, so exp underflows to exact 0
    ident_d = nc.inline_tensor(np.eye(128).astype(BF), name="ident")
    mneg_d = nc.inline_tensor(
        (-1e9 * np.tril(np.ones((128, 128)), -1)).astype(BF), name="mneg")

    with (tile.TileContext(nc) if for_sim else _SplitDrainTC(nc)) as tc:
        with tc.tile_pool(name="persist", bufs=1) as pp, \
             tc.tile_pool(name="big", bufs=2) as bigp, \
             tc.tile_pool(name="t32p", bufs=2) as t32p, \
             tc.tile_pool(name="frp", bufs=2) as frp, \
             tc.tile_pool(name="sqp", bufs=3) as sqp, \
             tc.tile_pool(name="ptp", bufs=4) as ptp, \
             tc.tile_pool(name="wp", bufs=4) as wp, \
             tc.tile_pool(name="ps", bufs=2, space="PSUM") as ps:

            # ---------------- constants / small inputs ----------------
            # const + weight DMAs ride the scalar-engine HWDGE queue so the
            # SP queue carries only the latency-critical x chunks
            ident_sb = pp.tile([128, 128], BF16, tag="ident")
            nc.scalar.dma_start(ident_sb[:], ident_d[:])
            mneg_sb = pp.tile([128, 128], BF16, tag="mneg")
            nc.scalar.dma_start(mneg_sb[:], mneg_d[:])
            ones512 = pp.tile([128, 512], BF16, tag="ones512")
            nc.vector.memset(ones512[:], 1.0)
            j32 = t32p.tile([128, 128], F32, tag="t32", name="j32")
            nc.vector.memset(j32[:], 1.0 / C)
            jr = pp.tile([128, 128], F32R, tag="jr")
            nc.vector.tensor_copy(jr[:], j32[:])
            o32 = t32p.tile([128, 128], F32, tag="t32", name="o32")
            nc.vector.memset(o32[:], 1.0)
            o64r = pp.tile([128, 128], F32R, tag="o64r")
            nc.vector.tensor_copy(o64r[:], o32[:])
            # x / x1 carry a WS (=64) scale on chip; var scales by WS^2, so
            # pre-scale eps to keep rstd = 1/(WS*std) exact through Ln/Exp
            eps_sb = pp.tile([128, 1], F32, tag="eps")
            nc.vector.memset(eps_sb[:], LN_EPS * WS * WS)

            bqksb = pp.tile([128, 2 * NC], F32, tag="bqksb")
            nc.scalar.dma_start(bqksb[:], bqk.rearrange("(c p) -> p c", p=128))
            bfcsb = pp.tile([128, NF], F32, tag="bfcsb")
            nc.scalar.dma_start(bfcsb[:], bfc.rearrange("(c p) -> p c", p=128))
            bvsb = pp.tile([1, C], BF16, tag="bvsb")
            nc.scalar.dma_start(bvsb[:], bv[:])
            bprojsb = pp.tile([128, NC], F32, tag="bprojsb")
            nc.scalar.dma_start(bprojsb[:], bproj.rearrange("(c p) -> p c", p=128))
            bfc2sb = pp.tile([128, NC], F32, tag="bfc2sb")
            nc.scalar.dma_start(bfc2sb[:], bfc2.rearrange("(c p) -> p c", p=128))

            import contextlib
            loop_cm = (tc.For_i(0, loop_n, 1) if loop_n
                       else contextlib.nullcontext())
            with loop_cm:
              for _rep in range(nrep):
                # ---------------- big persistent activations ----------------
                xts = bigp.tile([128, NC, T], F32R, tag="big")     # x^T (fp32r)
                xTv = xT.rearrange("(c p) t -> p c t", p=128)
                # x DMA: first token-half of every chunk first, then wv, then
                # second half, so LN1 half 0 completes as early as possible
                for c in range(NC):
                    nc.sync.dma_start(xts[:, c, 0:512], xTv[:, c, 0:512])
                wvsb = pp.tile([128, NC, C], F8, tag="wvsb")
                nc.scalar.dma_start(wvsb[:], wv.rearrange("(c p) v -> p c v", p=128))
                for c in range(NC):
                    nc.sync.dma_start(xts[:, c, 512:T], xTv[:, c, 512:T])

                x1 = pp.tile([128, NC, T], F32R, tag="x1")         # residual after attn
                lnout = pp.tile([128, NC, T], F8, tag="lnout")     # LN output (reused)
                qT = pp.tile([128, NC, T], BF16, tag="qT")
                kT = pp.tile([128, NC, T], BF16, tag="kT")
                yT = pp.tile([128, NC, T], F8, tag="yT")
                vsb = pp.tile([128, NT, H, HD + 1], BF16, tag="vsb")
                nc.vector.memset(vsb[:, :, :, HD:HD + 1], 1.0)
                rstd_sb = pp.tile([128, T], F32, tag="rstd")
                mu_sb = pp.tile([128, T], F32, tag="mu")

                # ---------------- LayerNorm helpers ----------------
                # stats: MU/SSQ [128, 2, 512] PSUM (Y tags), per (chunk, half)
                def ln_stats(src_c, MU, SSQ, c, h, tag):
                    hs = slice(512 * h, 512 * h + 512)
                    x2 = sqp.tile([128, 512], F32R, tag="sq",
                                  name=f"x2{tag}_{c}_{h}")
                    nc.scalar.activation(x2[:], src_c[:, hs], AF.Square)
                    nc.tensor.matmul(MU[:, h, :], jr[:], src_c[:, hs],
                                     start=(c == 0), stop=(c == NC - 1))
                    nc.tensor.matmul(SSQ[:, h, :], jr[:], x2[:],
                                     start=(c == 0), stop=(c == NC - 1))

                def ln_finalize(MU, SSQ, h, tag):
                    # rstd = exp(-0.5*ln(var+eps)); var = E[x^2] - mu^2
                    # h=None processes both halves in single [128,2,512] ops
                    sl = (slice(None) if h is None else
                          slice(h, h + 1))
                    hs = (slice(0, T) if h is None else
                          slice(512 * h, 512 * h + 512))
                    w = 2 if h is None else 1
                    musq = t32p.tile([128, 2, 512], F32, tag="th",
                                     name=f"musq{tag}{h}")
                    nc.scalar.activation(musq[:, 0:w, :], MU[:, sl, :],
                                         AF.Square)
                    var = t32p.tile([128, 2, 512], F32, tag="th",
                                    name=f"var{tag}{h}")
                    nc.vector.tensor_tensor(var[:, 0:w, :], SSQ[:, sl, :],
                                            musq[:, 0:w, :], op=OP.subtract)
                    lnv = t32p.tile([128, 2, 512], F32, tag="th",
                                    name=f"lnv{tag}{h}")
                    nc.scalar.activation(lnv[:, 0:w, :], var[:, 0:w, :],
                                         AF.Ln, bias=eps_sb[:])
                    nc.scalar.activation(
                        rstd_sb[:, hs].rearrange("p (h n) -> p h n", n=512),
                        lnv[:, 0:w, :], AF.Exp, scale=-0.5)
                    nc.vector.tensor_copy(
                        mu_sb[:, hs].rearrange("p (h n) -> p h n", n=512),
                        MU[:, sl, :])

                def ln_norm_chunk(src_c, dst, MU, c, h):
                    hs = slice(512 * h, 512 * h + 512)
                    cen = t32p.tile([128, 512], F32, tag="th",
                                    name=f"cen{h}_{c}_{id(src_c) % 97}")
                    nc.gpsimd.tensor_tensor(cen[:], src_c[:, hs],
                                            mu_sb[:, hs], op=OP.subtract)
                    nc.vector.tensor_tensor(dst[:, c, hs], cen[:],
                                            rstd_sb[:, hs], op=OP.mult)

                # ---------------- LN1, half-pipelined with x DMA ------------
                _mark(nc, 'ln1')
                MU1 = ps.tile([128, 2, 512], F32, tag="Y", name="MU1")
                SSQ1 = ps.tile([128, 2, 512], F32, tag="Y", name="SSQ1")
                for c in range(NC):
                    ln_stats(xts[:, c, :], MU1, SSQ1, c, 0, "a")
                ln_finalize(MU1, SSQ1, 0, "a")
                for c in range(NC):
                    ln_norm_chunk(xts[:, c, :], lnout, MU1, c, 0)

                def v_tile(ti):
                    # PSUM = WS*(x_n @ Wv + bv); ACT copy descales by 1/WS
                    pm = ps.tile([128, T], F32, tag="A", name=f"vp{ti}")
                    for k in range(NC // 2):
                        lh = lnout[:, 2 * k:2 * k + 2, ti * 128:(ti + 1) * 128]
                        nc.tensor.matmul(pm[:, 0:512], lh,
                                         wvsb[:, 2 * k:2 * k + 2, 0:512],
                                         start=(k == 0), stop=False,
                                         perf_mode=DR)
                        nc.tensor.matmul(pm[:, 512:768], lh,
                                         wvsb[:, 2 * k:2 * k + 2, 512:768],
                                         start=(k == 0), stop=False,
                                         perf_mode=DR)
                    nc.tensor.matmul(pm[:, 0:512], ones512[0:1, 0:128],
                                     bvsb[0:1, 0:512], start=False, stop=True)
                    nc.tensor.matmul(pm[:, 512:768], ones512[0:1, 0:128],
                                     bvsb[0:1, 512:768], start=False, stop=True)
                    nc.scalar.activation(
                        vsb[:, ti, :, 0:HD],
                        pm[:, 0:768].rearrange("p (h d) -> p h d", h=H),
                        AF.Copy, scale=1.0 / WS)

                _mark(nc, 'V')
                v_tile(0)
                v_tile(1)
                for c in range(NC):
                    ln_stats(xts[:, c, :], MU1, SSQ1, c, 1, "a")
                v_tile(2)
                v_tile(3)
                ln_finalize(MU1, SSQ1, 1, "a")
                for c in range(NC):
                    ln_norm_chunk(xts[:, c, :], lnout, MU1, c, 1)

                # ---------------- attention ----------------
                def qk_chunk(mi, split_evict=False):
                    # Q^T / K^T feature chunk (feature-major): lhsT = W tile.
                    # PSUM = WS*(x_n @ Wqk); evict as pm/WS + bqk in one
                    # dual-scalar DVE op.
                    pm = ps.tile([128, T], F32, tag="A", name=f"qkp{mi}")
                    wt = wp.tile([128, NC, 128], F8, tag="w", name=f"wqk{mi}")
                    nc.scalar.dma_start(wt[:], wqk[mi].rearrange("p (c m) -> p c m", c=NC))
                    for k in range(NC // 2):
                        for n0 in (0, 512):
                            nc.tensor.matmul(pm[:, n0:n0 + 512],
                                             wt[:, 2 * k:2 * k + 2, :],
                                             lnout[:, 2 * k:2 * k + 2,
                                                   n0:n0 + 512],
                                             start=(k == 0),
                                             stop=(k == NC // 2 - 1),
                                             perf_mode=DR)
                    dstt = qT if mi < NC else kT
                    nc.vector.tensor_scalar(
                        dstt[:, mi % NC, :], pm[:], 1.0 / WS,
                        bqksb[:, mi:mi + 1], op0=OP.mult, op1=OP.add)

                def attn_pair(p, yPa, yPb):
                    # two heads (2p: partitions 0:64, 2p+1: 64:128) interleaved
                    # per k-tile; for short k-tiles (qlen<=512) both heads'
                    # scores share one PSUM tile and one exp call.
                    ch = p
                    for kt in range(NT):
                        qs = 128 * kt
                        qlen = T - qs
                        sts, pts = [], []
                        if qlen <= 512:
                            sT2 = ps.tile([128, 2, 512], F32, tag="A",
                                          name=f"sT2{p}_{kt}")
                            pt2 = ptp.tile([128, 2, 512], BF16, tag="pt",
                                           name=f"pt2{p}_{kt}")
                            for i, p0 in enumerate((0, 64)):
                                lh = kT[p0:p0 + 64, ch, qs:qs + 128]
                                nc.tensor.matmul(sT2[:, i, 0:qlen], lh,
                                                 qT[p0:p0 + 64, ch, qs:T],
                                                 start=True, stop=False)
                                nc.tensor.matmul(sT2[:, i, 0:128], ident_sb[:],
                                                 mneg_sb[:], start=False,
                                                 stop=True)
                            nc.scalar.activation(pt2[:, :, 0:qlen],
                                                 sT2[:, :, 0:qlen], AF.Exp,
                                                 scale=float(1.0 / np.sqrt(HD)))
                            for i in range(2):
                                pts.append(pt2[:, i, :])
                        else:
                          for i, p0 in enumerate((0, 64)):
                            sT = ps.tile([128, T], F32, tag="A",
                                         name=f"sT{p}_{i}_{kt}")
                            lh = kT[p0:p0 + 64, ch, qs:qs + 128]
                            for a in range(0, qlen, 512):
                                bnd = min(a + 512, qlen)
                                nc.tensor.matmul(sT[:, a:bnd], lh,
                                                 qT[p0:p0 + 64, ch,
                                                    qs + a:qs + bnd],
                                                 start=True, stop=(a > 0))
                            nc.tensor.matmul(sT[:, 0:128], ident_sb[:],
                                             mneg_sb[:], start=False,
                                             stop=True)
                            sts.append(sT)
                          for i in range(2):
                            pt = ptp.tile([128, T], BF16, tag="pt",
                                          name=f"pt{p}_{i}_{kt}")
                            nc.scalar.activation(pt[:, 0:qlen], sts[i][:, 0:qlen],
                                                 AF.Exp,
                                                 scale=float(1.0 / np.sqrt(HD)))
                            pts.append(pt)
                        for i, yP in enumerate((yPa, yPb)):
                            vl = vsb[:, kt, 2 * p + i, :]
                            pt = pts[i]
                            if qs < 512:
                                nc.tensor.matmul(yP[0:HD + 1, qs:512], vl,
                                                 pt[:, 0:512 - qs],
                                                 start=(kt == 0), stop=(kt == 3))
                                nc.tensor.matmul(yP[0:HD + 1, 512:T], vl,
                                                 pt[:, 512 - qs:T - qs],
                                                 start=(kt == 0),
                                                 stop=(kt == NT - 1))
                            else:
                                nc.tensor.matmul(yP[0:HD + 1, qs:T], vl,
                                                 pt[:, 0:qlen],
                                                 start=False, stop=(kt == NT - 1))

                def attn_norm_pre(p, yPa, yPb):
                    return ()

                def attn_norm_post(p, yPa, yPb):
                    # per-head: rec = exp(-ln(den)); bcast via K=1 fp32r
                    # matmul; y = y' * rec (baseline-proven form)
                    for i, yP in enumerate((yPa, yPb)):
                        h = 2 * p + i
                        ch = p
                        lnden = t32p.tile([128, T], F32, tag="t32",
                                          name=f"lnden{h}")
                        nc.scalar.activation(lnden[HD:HD + 1, :],
                                             yP[HD:HD + 1, :], AF.Ln)
                        rcr = frp.tile([128, T], F32R, tag="fr", name=f"rc{h}")
                        nc.scalar.activation(rcr[HD:HD + 1, :],
                                             lnden[HD:HD + 1, :], AF.Exp,
                                             scale=-1.0)
                        R64 = ps.tile([128, T], F32, tag="A", name=f"r64_{h}")
                        for n0 in (0, 512):
                            nc.tensor.matmul(R64[0:64, n0:n0 + 512],
                                             o64r[HD:HD + 1, 0:64],
                                             rcr[HD:HD + 1, n0:n0 + 512],
                                             start=True, stop=True)
                        r64sb = t32p.tile([128, T], F32, tag="t32",
                                          name=f"r64sb{h}")
                        nc.vector.tensor_copy(r64sb[0:64, :], R64[0:64, :])
                        if i == 0:
                            nc.vector.tensor_tensor(yT[0:64, ch, :],
                                                    yP[0:64, :],
                                                    r64sb[0:64, :], op=OP.mult)
                        else:
                            yo = ptp.tile([128, T], F8, tag="pt",
                                          name=f"yo{h}")
                            nc.vector.tensor_tensor(yo[0:64, :], yP[0:64, :],
                                                    r64sb[0:64, :], op=OP.mult)
                            nc.sync.dma_start(yT[64:128, ch, :], yo[0:64, :])

                # interleave: QK chunk pair p, then attention heads 2p / 2p+1 —
                # the next pair's QK matmuls keep PE busy while ACT runs exp.
                # qk chunk 0 runs before the second V half so attention starts
                # as soon as V tiles land.
                _mark(nc, 'pair0')
                qk_chunk(NC)
                qk_chunk(0, split_evict=True)
                for ti in range(4, NT):
                    v_tile(ti)
                pending = None
                for p in range(NC):
                    if p > 0:
                        _mark(nc, f'pair{p}')
                        if pending is not None:
                            pre = attn_norm_pre(*pending)
                        qk_chunk(NC + p)  # K chunk p
                        qk_chunk(p, split_evict=True)  # Q chunk p
                        # normalization of the previous pair lands here so its
                        # Ln/Exp chain overlaps this pair's QK matmuls on PE
                        if pending is not None:
                            attn_norm_post(*pending)
                    yPa = ps.tile([128, T], F32, tag="Y", name=f"yp{2 * p}")
                    yPb = ps.tile([128, T], F32, tag="Y", name=f"yp{2 * p + 1}")
                    attn_pair(p, yPa, yPb)
                    pending = (p, yPa, yPb)
                if pending is not None:
                    pre = attn_norm_pre(*pending)
                    attn_norm_post(*pending)

                # ---------------- proj with fused LN2 stats ----------------
                _mark(nc, 'proj')
                MU2 = ps.tile([128, 2, 512], F32, tag="Y", name="MU2")
                SSQ2 = ps.tile([128, 2, 512], F32, tag="Y", name="SSQ2")
                for m in range(NC):
                    # PSUM = WS*(y @ Wp); x1 stays WS-scaled (bproj is host-
                    # scaled, xts already WS-scaled), so eviction is unchanged
                    pm = ps.tile([128, T], F32, tag="A", name=f"pj{m}")
                    wt = wp.tile([128, NC, 128], F8, tag="w", name=f"wpj{m}")
                    nc.scalar.dma_start(wt[:], wproj[m].rearrange("p (c m) -> p c m", c=NC))
                    for k in range(NC // 2):
                        for n0 in (0, 512):
                            nc.tensor.matmul(pm[:, n0:n0 + 512],
                                             wt[:, 2 * k:2 * k + 2, :],
                                             yT[:, 2 * k:2 * k + 2, n0:n0 + 512],
                                             start=(k == 0),
                                             stop=(k == NC // 2 - 1),
                                             perf_mode=DR)
                    # x1 = (pm + bproj) + x, halves on DVE + Pool in parallel
                    # so the fused LN2 stats start as soon as possible
                    for h in (0, 1):
                        hs = slice(512 * h, 512 * h + 512)
                        nc.vector.scalar_tensor_tensor(
                            x1[:, m, hs], pm[:, hs], bprojsb[:, m:m + 1],
                            xts[:, m, hs], op0=OP.add, op1=OP.add)
                        ln_stats(x1[:, m, :], MU2, SSQ2, m, h, "b")

                # half 0 finalizes + normalizes first; fc1 (n0-major) starts
                # on half-0 columns while half 1 is still normalizing
                _mark(nc, 'ln2')
                ln_finalize(MU2, SSQ2, 0, "b")
                for c in range(NC):
                    ln_norm_chunk(x1[:, c, :], lnout, MU2, c, 0)
                ln_finalize(MU2, SSQ2, 1, "b")
                for c in range(NC):
                    ln_norm_chunk(x1[:, c, :], lnout, MU2, c, 1)

                # ---------------- MLP ----------------
                for half in range(2):
                    _mark(nc, f'mlp{half}')
                    h2 = bigp.tile([128, NF // 2, T], F8, tag="big",
                                   name=f"h2_{half}")
                    # software-pipelined halves: h1 of tile m is emitted 3
                    # tiles after h0, so half-0 matmuls keep PE busy while
                    # LN2 half 1 finalizes (half == 1 of the MLP reuses the
                    # same schedule harmlessly). wt lives h0(m)..h1(m): <= 4
                    # concurrent tiles, matching the wp ring.
                    LAG = 3
                    wts = {}
                    sched = []
                    for m in range(NF // 2 + LAG):
                        if m < NF // 2:
                            sched.append((m, 0))
                        if m >= LAG:
                            sched.append((m - LAG, 1))
                    for m, hh in sched:
                        mg = half * (NF // 2) + m
                        if hh == 0:
                            wt = wp.tile([128, NC, 128], F8, tag="w",
                                         name=f"wfc{mg}")
                            nc.scalar.dma_start(
                                wt[:], wfc[mg].rearrange("p (c m) -> p c m",
                                                         c=NC))
                            wts[m] = wt
                        wt = wts[m]
                        n0 = 512 * hh
                        pm = ps.tile([128, T], F32, tag="A", name=f"fc{mg}_{hh}")
                        for k in range(NC // 2):
                            nc.tensor.matmul(pm[:, n0:n0 + 512],
                                             wt[:, 2 * k:2 * k + 2, :],
                                             lnout[:, 2 * k:2 * k + 2,
                                                   n0:n0 + 512],
                                             start=(k == 0),
                                             stop=(k == NC // 2 - 1),
                                             perf_mode=DR)
                        # h2 = gelu(PSUM/WS + bfc) written as e4m3 (true scale)
                        nc.scalar.activation(h2[:, m, n0:n0 + 512],
                                             pm[:, n0:n0 + 512], AF.Gelu,
                                             bias=bfcsb[:, mg:mg + 1],
                                             scale=1.0 / WS)
                    for m in range(NC):
                        # PSUM = WS*(h @ Wfc2); x1 accumulates in WS-scaled
                        # domain (bfc2 host-scaled); host divides out by WS
                        pm = ps.tile([128, T], F32, tag="A", name=f"fc2_{half}_{m}")
                        wt = wp.tile([128, NF // 2, 128], F8, tag="w",
                                     name=f"wfc2_{half}_{m}")
                        nc.scalar.dma_start(
                            wt[:],
                            wfc2[m, :, half * 1536:(half + 1) * 1536]
                            .rearrange("p (c m) -> p c m", c=NF // 2))
                        last = NF // 4 - 1
                        for k in range(NF // 4):
                            for n0 in (0, 512):
                                nc.tensor.matmul(pm[:, n0:n0 + 512],
                                                 wt[:, 2 * k:2 * k + 2, :],
                                                 h2[:, 2 * k:2 * k + 2,
                                                    n0:n0 + 512],
                                                 start=(k == 0),
                                                 stop=(k == last),
                                                 perf_mode=DR)
                        # x1 = (pm + bias) + x1 in one fused DVE op
                        bias = bfc2sb[:, m:m + 1] if half == 0 else 0.0
                        nc.vector.scalar_tensor_tensor(
                            x1[:, m, :], pm[:], bias, x1[:, m, :],
                            op0=OP.add, op1=OP.add)
                        if half == 1:
                            # gpsimd SWDGE: keeps the SP queue free so the
                            # next iteration's x chunks prefetch during MLP.
                            # (SWDGE inside a For_i hardware loop breaks
                            # walrus codegen -- loop timing builds use the
                            # scalar HWDGE queue instead.)
                            eng = nc.scalar if loop_n else nc.gpsimd
                            eng.dma_start(outT[m * 128:(m + 1) * 128, :],
                                          x1[:, m, :])

    if not for_sim:
        _split_excess_waits(nc)
    return nc


_STATE = {}


def _prepare(inputs):
    x = np.asarray(inputs["x"], np.float32)
    ln1_g = np.asarray(inputs["ln1_g"], np.float32)
    ln1_b = np.asarray(inputs["ln1_b"], np.float32)
    ln2_g = np.asarray(inputs["ln2_g"], np.float32)
    ln2_b = np.asarray(inputs["ln2_b"], np.float32)
    W_attn = np.asarray(inputs["W_attn"], np.float32)
    b_attn = np.asarray(inputs["b_attn"], np.float32)
    W_proj = np.asarray(inputs["W_proj"], np.float32)
    b_proj = np.asarray(inputs["b_proj"], np.float32)
    W_fc = np.asarray(inputs["W_fc"], np.float32)
    b_fc = np.asarray(inputs["b_fc"], np.float32)
    W_fc2 = np.asarray(inputs["W_fc2"], np.float32)
    b_fc2 = np.asarray(inputs["b_fc2"], np.float32)

    # fold LN affine into the following matmul (exact): (n*g + b) @ W
    Wa = W_attn * ln1_g[:, None]
    ba = b_attn + ln1_b @ W_attn
    Wf = W_fc * ln2_g[:, None]
    bf = b_fc + ln2_b @ W_fc

    def blk(w):
        # [K, M] -> [M/128 blocks][128 kp][K/128 * 128 mp] with feature
        # f = 128*kc + kp on the partition axis; WS-scaled e4m3 so small
        # uniform weights land in the fp8 normal range
        K, M = w.shape
        return np.ascontiguousarray(
            (w * WS).astype(E4).reshape(K // 128, 128, M // 128, 128)
            .transpose(2, 1, 0, 3).reshape(M // 128, 128, K))

    shared = {
        "wqk": blk(Wa[:, :2 * C]),
        "wv": np.ascontiguousarray((Wa[:, 2 * C:] * WS).astype(E4)),
        "bqk": np.ascontiguousarray(ba[:2 * C]),
        "bv": np.ascontiguousarray((ba[None, 2 * C:] * WS).astype(BF)),
        "wproj": blk(W_proj),
        "bproj": np.ascontiguousarray(b_proj * WS),
        "wfc": blk(Wf),
        "bfc": np.ascontiguousarray(bf),
        "wfc2": blk(W_fc2),
        "bfc2": np.ascontiguousarray(b_fc2 * WS),
    }
    in_maps = []
    for b in range(B):
        m = dict(shared)
        m["xT"] = np.ascontiguousarray(x[b].T * WS)
        in_maps.append(m)
    return in_maps


def kernel(**inputs):
    in_maps = _prepare(inputs)
    if "nc" not in _STATE:
        _STATE["nc"] = _build()
    global _last_in_maps
    _last_in_maps = in_maps
    res = run_bass_kernel_spmd(_STATE["nc"], in_maps, core_ids=list(range(B)))
    out = np.stack([r["outT"].T for r in res.results]) * (1.0 / WS)
    return np.ascontiguousarray(out, dtype=np.float32)

